# revision 45
# baseline (speedup 1.0000x reference)
"""Multi-head attention (B=2, S=2048, E=1024, H=16, D=64) on 8 TRN2 cores.

Sharding: tensor-parallel over heads. Core c owns heads {2c, 2c+1}:
  - Q/K/V projections column-sharded (128 cols each per core)
  - attention for the core's 2 heads (both batches)
  - out-projection row-sharded (128 rows of Wo) -> partial [4096,1024] f16
  - host sums the 8 partials and adds bo.

On-chip layout (everything "transposed"):
  - xT [1024, 4096] (E-major, fp16) is prefetched ONCE into SBUF in large
    contiguous chunks spread over four DMA queues (sync/scalar/vector/
    gpsimd), so no compute ever waits on an x load after ~2us
  - projections produce Q^T, K^T [128, 4096] (head-dim on partitions) and
    V^T, which is PE-transposed to token-major V tiles
  - scores are computed transposed: scores^T[kk, q] so softmax's key
    reduction can ride the attn@V matmul (ones-column in V) and the
    key-padding mask folds into the exp() per-partition bias
  - attn@V emits Y^T directly into a single stacked [128, M] tile
    (head 0 on partitions 0-63, head 1 on 64-127), so the out-projection
    is ONE K=128 matmul per tile instead of two K=64 ones
  - out partials are written f16 (halves the 16MB DRAM write), summed on
    host in float64.

Perf notes:
  - matmul inputs fp16 (full PE rate); accumulation fp32 in PSUM
  - matmul cost is streamed-columns only, so the schedule keeps the PE
    saturated and ACT (exp) just under it: exp() batched [128,1024],
    normalization on DVE+DMA (stride-0 partition-broadcast DMA), each
    pass's normalization emitted one pass late
  - deferred batch-1 projections: two m-chunks run right after phase 1,
    two are interleaved into attention passes 0-3 so the per-pass PE
    load matches the exp() pace instead of front-loading it
  - out-proj tiles interleave into passes 4-7; out writes ride the
    gpsimd DMA queue (idle engine -> free trigger slots)
"""

import os
import numpy as np

B, S, E, H, D = 2, 2048, 1024, 16, 64
M = B * S            # 4096 tokens
P = 128              # partitions
NCORES = 8
KC = E // P          # 8 contraction chunks for projections
MCH = 512            # token chunk for projections
QCH = 512            # query chunk for attention
NQC = S // QCH       # 4 query chunks per batch
NKT = S // P         # 16 key tiles per batch
NEG = -1.0e30

LAST_RESULTS = None  # BassKernelResults of the most recent run (for test harness)
_PROGRAM = None


def _build_program():
    import concourse.bass as bass
    import concourse.tile as tile
    from concourse import bacc, mybir
    from concourse.masks import make_identity

    f32 = mybir.dt.float32
    f16 = mybir.dt.float16

    nc = bacc.Bacc(
        "TRN2",
        target_bir_lowering=False,
        debug=False,
        enable_asserts=False,
        num_devices=NCORES,
    )

    xT_d = nc.dram_tensor("xT", (E, M), f16, kind="ExternalInput").ap()
    wq_d = nc.dram_tensor("wq", (P, KC, P), f16, kind="ExternalInput").ap()
    wk_d = nc.dram_tensor("wk", (P, KC, P), f16, kind="ExternalInput").ap()
    wv_d = nc.dram_tensor("wv", (P, KC, P), f16, kind="ExternalInput").ap()
    wo_d = nc.dram_tensor("wo", (P, E), f16, kind="ExternalInput").ap()
    bq_d = nc.dram_tensor("bq", (P, 1), f32, kind="ExternalInput").ap()
    bk_d = nc.dram_tensor("bk", (P, 1), f32, kind="ExternalInput").ap()
    bv_d = nc.dram_tensor("bv", (P, 1), f32, kind="ExternalInput").ap()
    maskT_d = nc.dram_tensor("maskT", (P, B * 16), f32, kind="ExternalInput").ap()
    out_d = nc.dram_tensor("out", (M, E), f16, kind="ExternalOutput").ap()
    rsc_d = nc.dram_tensor("rscratch", (16, QCH), f32, kind="Internal").ap()

    with tile.TileContext(nc) as tc:
        with (
            tc.tile_pool(name="consts", bufs=1) as consts,
            tc.tile_pool(name="big", bufs=1) as big,
            tc.tile_pool(name="vt_pool", bufs=2) as vt_pool,
            tc.tile_pool(name="pt_pool", bufs=8) as pt_pool,
            tc.tile_pool(name="r_pool", bufs=2) as r_pool,
            tc.tile_pool(name="out_pool", bufs=6) as out_pool,
        ):
            # ---- constants ----
            wq_sb = consts.tile([P, KC, P], f16)
            wk_sb = consts.tile([P, KC, P], f16)
            wv_sb = consts.tile([P, KC, P], f16)
            wo_sb = consts.tile([P, E], f16)
            bq_sb = consts.tile([P, 1], f32)
            bk_sb = consts.tile([P, 1], f32)
            bv_sb = consts.tile([P, 1], f32)
            mask_sb = consts.tile([P, B * 16], f32)
            ident = consts.tile([P, P], f16)
            ones_h = consts.tile([P, M // P], f16)

            # ---- resident x^T [128, KC, M]: 64KB/partition ----
            xsb = big.tile([P, KC, M], f16)

            # Prefetch: first weights + batch-0 x chunks round-robin over the
            # four DMA queues so nothing downstream waits on HBM.
            # CRITICAL: a dma_start on a backed-up DGE queue BLOCKS the
            # issuing engine's sequencer until a descriptor slot frees. The
            # 8MB x prefetch saturates its queues for ~30us, so x rides ONLY
            # sync+gpsimd (engines with no early work). ACT (scalar queue)
            # gets just the tiny consts, staying free for phase-1 staging.
            # weights split per-kc so the first phase-1 matmul only waits on
            # one 32KB piece per weight (~6us) instead of full 256KB loads
            for kc in range(KC):
                nc.sync.dma_start(wq_sb[:, kc, :], wq_d[:, kc, :])
                nc.scalar.dma_start(wk_sb[:, kc, :], wk_d[:, kc, :])
                nc.scalar.dma_start(wv_sb[:, kc, :], wv_d[:, kc, :])
            nc.scalar.dma_start(bq_sb, bq_d)
            nc.scalar.dma_start(bk_sb, bk_d)
            nc.scalar.dma_start(bv_sb, bv_d)
            nc.scalar.dma_start(mask_sb, maskT_d)
            nc.scalar.dma_start(wo_sb, wo_d)
            # gpsimd engine work must precede its x triggers (queue blocking)
            make_identity(nc, ident)
            nc.vector.memset(ones_h, 1.0)
            # x in m-chunk-sized pieces so phase-1 starts on the first piece
            # and the two queues stay ahead of the PE's kc-loop
            qs = [nc.sync, nc.gpsimd]
            di = 0
            for mc in range(M // MCH):
                msl = bass.ts(mc, MCH)
                for kc in range(KC):
                    qs[(di + 1) % 2].dma_start(
                        xsb[:, kc, msl], xT_d[bass.ts(kc, P), msl]
                    )
                    di += 1

            # ---- big persistent activations ----
            QT = big.tile([P, M], f16)       # Q^T: head-dims on partitions
            KT = big.tile([P, M], f16)
            # token-major V tiles: [tok, mt, 2*(64 cols + ones col)]
            Vtm = big.tile([P, M // P, 2 * (D + 1)], f16)
            YT = big.tile([P, M], f16)       # stacked attention output^T

            ones_col = ones_h[:, 0 : M // P].rearrange("p (a b) -> p a b", b=1)
            nc.vector.tensor_copy(Vtm[:, :, D : D + 1], ones_col)
            nc.vector.tensor_copy(Vtm[:, :, 2 * D + 1 : 2 * D + 2], ones_col)

            Exp = mybir.ActivationFunctionType.Exp
            Ident = mybir.ActivationFunctionType.Identity

            def emit_vt_tiles(mc, vt, psum_pool, tag):
                for j in range(MCH // P):
                    mt = mc * (MCH // P) + j
                    vtp = psum_pool.tile([P, P], f16, tag=tag, bufs=2, name=tag)
                    nc.tensor.transpose(vtp, vt[:, bass.ts(j, P)], ident)
                    nc.vector.tensor_copy(Vtm[:, mt, 0:D], vtp[:, 0:D])
                    nc.vector.tensor_copy(
                        Vtm[:, mt, D + 1 : 2 * D + 1], vtp[:, D : 2 * D]
                    )

            def make_proj_units(mc_pair, pool, tag, vtp_tag, stage_eng):
                # two m-chunks processed per weight load (the serialized
                # ldweights on a stationary switch is ~95ns; share it)
                units = []
                mcs = [(mc, bass.ts(mc, MCH)) for mc in mc_pair]
                state = {}

                def stage(dst, src, b_sb):
                    if stage_eng == "act":
                        nc.scalar.activation(dst, src, Ident, bias=b_sb)
                    else:
                        nc.vector.tensor_scalar_add(dst, src, b_sb)

                def u_proj(w_sb, which):
                    ps = [pool.tile([P, MCH], f32, tag=tag, name="pp2")
                          for _ in range(2)]
                    for kc in range(KC):
                        for i in range(2):
                            nc.tensor.matmul(
                                ps[i], w_sb[:, kc, :], xsb[:, kc, mcs[i][1]],
                                start=(kc == 0), stop=(kc == KC - 1),
                            )
                    state[which] = ps

                def u_q_mm():
                    u_proj(wq_sb, "q")

                def u_q_st():
                    for i in range(2):
                        stage(QT[:, mcs[i][1]], state["q"][i], bq_sb)

                def u_k_mm():
                    u_proj(wk_sb, "k")

                def u_k_st():
                    for i in range(2):
                        stage(KT[:, mcs[i][1]], state["k"][i], bk_sb)

                def u_v_mm():
                    u_proj(wv_sb, "v")

                def u_v_st():
                    vts = []
                    for i in range(2):
                        vt = vt_pool.tile([P, MCH], f16, name="vt2", tag="vt2")
                        stage(vt, state["v"][i], bv_sb)
                        vts.append(vt)
                    state["vts"] = vts

                def u_t0():
                    emit_vt_tiles(mc_pair[0], state["vts"][0], pool, vtp_tag)

                def u_t1():
                    emit_vt_tiles(mc_pair[1], state["vts"][1], pool, vtp_tag)

                units += [u_q_mm, u_q_st, u_k_mm, u_k_st,
                          u_v_mm, u_v_st, u_t0, u_t1]
                return units

            pre_fill = []   # deferred batch-0 V transposes (run in pass 0)
            holders = {}    # late-bound pool refs for deferred closures
            with (
                tc.tile_pool(name="psum_p1", bufs=6, space="PSUM") as psum_p1,
            ):
                # ---- phase 1: batch-0 projections, kc-outer within each
                # pair of m-chunks so compute starts on the first x chunk ----
                for grp in range(S // (2 * MCH)):
                    psums = []
                    for half in range(2):
                        mc = 2 * grp + half
                        msl = bass.ts(mc, MCH)
                        qp = psum_p1.tile([P, MCH], f32, tag="p1", name="qp")
                        kp = psum_p1.tile([P, MCH], f32, tag="p1", name="kp")
                        vp = psum_p1.tile([P, MCH], f32, tag="p1", name="vp")
                        psums.append((msl, qp, kp, vp))
                    for kc in range(KC):
                        st, sp = kc == 0, kc == KC - 1
                        for wi, w_sb in ((1, wq_sb), (2, wk_sb), (3, wv_sb)):
                            for half in range(2):
                                nc.tensor.matmul(
                                    psums[half][wi], w_sb[:, kc, :],
                                    xsb[:, kc, psums[half][0]],
                                    start=st, stop=sp,
                                )
                    for half in range(2):
                        msl, qp, kp, vp = psums[half]
                        mc = 2 * grp + half
                        # ACT is idle pre-attention: stage psum->sbuf there
                        nc.scalar.activation(QT[:, msl], qp, Ident, bias=bq_sb)
                        nc.scalar.activation(KT[:, msl], kp, Ident, bias=bk_sb)
                        vt = vt_pool.tile([P, MCH], f16, name="vt", bufs=4)
                        nc.scalar.activation(vt, vp, Ident, bias=bv_sb)
                        if mc == 0:
                            emit_vt_tiles(mc, vt, psum_p1, "vtp")
                        else:
                            # defer into pass-0 fillers: shaves ~5us off the
                            # serial head (only m-chunk-0 tiles are needed at
                            # pass-0 kt=0)
                            pre_fill.append(
                                lambda mc=mc, vt=vt: emit_vt_tiles(
                                    mc, vt, holders["op"], "op"
                                )
                            )

                # batch-1 projections are NOT run here: their x is still in
                # flight at phase-1 end; they interleave into passes 0-3

            # ---- phase 2: attention, deferred normalization, out-proj ----
            with (
                tc.tile_pool(name="psum_sc", bufs=2, space="PSUM") as psum_sc,
                tc.tile_pool(name="psum_av", bufs=2, space="PSUM") as psum_av,
                tc.tile_pool(name="psum_op", bufs=2, space="PSUM") as psum_op,
            ):
                if True:
                    norm_idx = [0]

                    def psum_to_sbuf(dst, src):
                        # DVE only: ACT must stay a pure-exp stream during
                        # attention or its stalls starve the PE
                        nc.vector.tensor_copy(dst, src)

                    def norm_dma(qi, av_sb):
                        # partition-broadcast the raw sums [1,512] -> [64,512]
                        # via DRAM bounce (SBUF-source DMAs cannot have a zero
                        # partition step)
                        qisl = bass.ts(qi, QCH)
                        ni = norm_idx[0]
                        norm_idx[0] += 1
                        nc.sync.dma_start(rsc_d[ni, :], av_sb[D : D + 1, qisl])
                        sb = r_pool.tile([D, QCH], f32, tag="sb", bufs=4, name="sb")
                        src = rsc_d[ni : ni + 1, :]
                        src_b = bass.AP(
                            tensor=src.tensor,
                            offset=src.offset,
                            ap=[[0, D]] + [list(x) for x in src.ap[1:]],
                        )
                        nc.sync.dma_start(sb, src_b)
                        return sb

                    def norm_dve(b, h, pr, qi, av_sb, sb):
                        # reciprocal at base partition 0 (custom-DVE approx
                        # ops misbehave at base 64), then scale into YT
                        qc = 2 * pr + qi
                        qsl = bass.ds(b * S + qc * QCH, QCH)
                        qisl = bass.ts(qi, QCH)
                        rbs = r_pool.tile([D, QCH], f32, tag="rbs", bufs=4, name="rbs")
                        # fast approx reciprocal: ~1e-3 rel err, well inside
                        # the tolerance, and 1 DVE instr instead of 3
                        nc.vector.reciprocal_approx_fast(rbs, sb)
                        nc.vector.tensor_mul(
                            YT[D * h : D * (h + 1), qsl], av_sb[0:D, qisl], rbs
                        )

                    def emit_norm_qc(b, h, pr, qi, av_sb):
                        sb = norm_dma(qi, av_sb)
                        norm_dve(b, h, pr, qi, av_sb, sb)

                    def emit_norm(b, h, pr, av_sb):
                        for qi in range(2):
                            emit_norm_qc(b, h, pr, qi, av_sb)

                    tail_mode = [False]

                    def emit_outproj_tile(b, j, spare_psum=False):
                        m0 = b * S + j * P
                        for ec in range(E // 512):
                            esl = bass.ts(ec, 512)
                            if spare_psum and ec % 2 == 1:
                                # after the last pass the score banks are free
                                op = psum_sc.tile([P, 512], f32, tag="sc", name="op_s")
                            else:
                                op = psum_op.tile([P, 512], f32, tag="op", name="op")
                            nc.tensor.matmul(
                                op, YT[:, bass.ds(m0, P)], wo_sb[:, esl],
                                start=True, stop=True,
                            )
                            osb = out_pool.tile([P, 512], f16, name="osb")
                            if (spare_psum or tail_mode[0]) and ec % 2 == 1:
                                # split tail staging across ACT and DVE so
                                # neither serializes the drain
                                nc.scalar.copy(osb, op)
                            else:
                                psum_to_sbuf(osb, op)
                            # tail: sync queue is reserved for norm bounces
                            oq = nc.gpsimd if (ec == 0 or tail_mode[0]) else nc.sync
                            oq.dma_start(out_d[bass.ds(m0, P), esl], osb)

                    def emit_outproj(b, jlo, jhi, spare_psum=False):
                        for j in range(jlo, jhi):
                            emit_outproj_tile(b, j, spare_psum)

                    passes = [
                        (b, h, pr) for b in range(B) for h in range(2) for pr in range(2)
                    ]
                    holders["op"] = psum_op
                    pending = []
                    filler = pre_fill
                    filler += list(make_proj_units([4, 5], psum_op, "op", "op", "dve"))
                    filler += list(make_proj_units([6, 7], psum_op, "op", "op", "dve"))
                    for pi, (b, h, pr) in enumerate(passes):
                        if pi == 4:
                            # deferred projections must be fully emitted before
                            # any batch-1 read (emission order defines dataflow)
                            while filler:
                                filler.pop(0)()
                            filler = [
                                (lambda b0=0, j0=j: emit_outproj_tile(b0, j0))
                                for j in range(S // P)
                            ]
                        dsl = bass.ds(D * h, D)
                        av2 = psum_av.tile(
                            [D + 1, 2 * QCH], f32, tag="av", bufs=1, name="av"
                        )

                        def av_pair(t, pt):
                            lhs_v = Vtm[:, b * 16 + t, h * (D + 1) : (h + 1) * (D + 1)]
                            for qi in range(2):
                                nc.tensor.matmul(
                                    av2[:, bass.ts(qi, QCH)], lhs_v,
                                    pt[:, bass.ts(qi, QCH)],
                                    start=(t == 0), stop=(t == NKT - 1),
                                )

                        # software-pipelined one kt deep: scores(t) and exp(t)
                        # are issued before av(t-1), so the PE's av never waits
                        # on the exp it consumes
                        prev_pt = None
                        for t in range(NKT):
                            ksl = bass.ds(b * S + t * P, P)
                            lhs_k = KT[dsl, ksl]
                            sc2 = psum_sc.tile([P, 2 * QCH], f32, tag="sc", name="sc2")
                            for qi in range(2):
                                qc = 2 * pr + qi
                                qsl = bass.ds(b * S + qc * QCH, QCH)
                                nc.tensor.matmul(
                                    sc2[:, bass.ts(qi, QCH)], lhs_k, QT[dsl, qsl],
                                    start=True, stop=True,
                                )
                            pt = pt_pool.tile([P, 2 * QCH], f16, tag="pt", name="pt")
                            bt = b * 16 + t
                            nc.scalar.activation(
                                pt, sc2, Exp, bias=mask_sb[:, bt : bt + 1], scale=1.0
                            )
                            if prev_pt is not None:
                                av_pair(t - 1, prev_pt)
                            prev_pt = pt
                            # early-emit the previous pass's normalization
                            # (DVE/DMA only) so its reciprocal never gates PE
                            if t == 1 and pending:
                                emit_norm(*pending.pop(0))
                                if pi == 7:
                                    # batch-1 pair-0 columns are now normalized
                                    filler.extend(
                                        (lambda b1=1, j1=j: emit_outproj_tile(b1, j1))
                                        for j in range(S // (2 * P))
                                    )
                            # interleave independent PE work (deferred
                            # projections, ready out-proj tiles); cadence per
                            # pass keeps each pass near the exp() pace.
                            # pass 0 starts late (batch-1 x still landing);
                            # pass 7 keeps 4 tiles in reserve for the tail
                            # norm bubble
                            if filler:
                                if pi == 0:
                                    if t in (1, 5, 9, 11, 13, 15):
                                        filler.pop(0)()
                                elif pi < 4:
                                    if t in (3, 7, 11, 15):
                                        filler.pop(0)()
                                elif pi == 7:
                                    # keep the pair-0 tiles in reserve: they
                                    # fill the PE during the tail norm's DRAM
                                    # round trip
                                    if t == 5:
                                        filler.pop(0)()
                                elif pi == 6:
                                    if t in (2, 6, 10, 14):
                                        filler.pop(0)()
                                elif t % 3 == 2:
                                    filler.pop(0)()
                        av_pair(NKT - 1, prev_pt)
                        # stage accumulators to SBUF, freeing the PSUM banks
                        av_sb = r_pool.tile(
                            [D + 1, 2 * QCH], f32, tag="avsb", bufs=3, name="avsb"
                        )
                        if pi == 7:
                            # last pass: denominator row staged separately on
                            # ACT (idle now) so the tail norm's DRAM bounce
                            # launches ~1.5us earlier
                            nc.scalar.copy(av_sb[D : D + 1, :], av2[D : D + 1, :])
                            nc.vector.tensor_copy(av_sb[0:D, :], av2[0:D, :])
                        else:
                            psum_to_sbuf(av_sb, av2)
                        pending.append((b, h, pr, av_sb))
                    # tail: drain remaining filler; interleave the last norm
                    # per-query-chunk with the out-proj tiles it unblocks
                    # tail: the last pass's denominator rows bounce straight
                    # from PSUM (av2 still live) so the broadcast RTT starts
                    # ~1.5us earlier; the reserved out-proj tiles fill the PE
                    # while the RTT is in flight
                    # tail order matters: both norm DMA bounces launch first,
                    # then the reserved out-proj tiles fill the PE (and the
                    # in-order DVE queue) while the DRAM round-trip is in
                    # flight, then the norms' DVE work and the final tiles
                    b_l, h_l, pr_l, av_sb_l = pending.pop(0)   # (1,1,1)
                    tail_mode[0] = True
                    sb0 = norm_dma(0, av_sb_l)
                    sb1 = norm_dma(1, av_sb_l)
                    while filler:
                        filler.pop(0)()
                    norm_dve(b_l, h_l, pr_l, 0, av_sb_l, sb0)
                    emit_outproj(1, 2 * S // (4 * P), 3 * S // (4 * P), spare_psum=True)
                    norm_dve(b_l, h_l, pr_l, 1, av_sb_l, sb1)
                    emit_outproj(1, 3 * S // (4 * P), S // P, spare_psum=True)

    nc.compile()
    return nc


def kernel(x, mask, Wq, bq, Wk, bk, Wv, bv, Wo, bo):
    global LAST_RESULTS, _PROGRAM
    from concourse.bass_utils import run_bass_kernel_spmd

    if _PROGRAM is None:
        _PROGRAM = _build_program()
    nc = _PROGRAM

    f16 = np.float16
    x = np.asarray(x, dtype=np.float32)
    mask = np.asarray(mask)
    f32c = lambda a: np.ascontiguousarray(np.asarray(a, dtype=np.float32))

    xT = np.ascontiguousarray(x.reshape(M, E).T.astype(f16))     # [E, M]
    maskf = np.where(mask, np.float32(NEG), np.float32(0.0)).astype(np.float32)
    maskT = np.ascontiguousarray(
        maskf.reshape(B, 16, P).transpose(2, 0, 1).reshape(P, B * 16)
    )
    scale = np.float32(1.0 / np.sqrt(D))

    in_maps = []
    for c in range(NCORES):
        csl = slice(P * c, P * (c + 1))
        wq_c = (np.asarray(Wq, dtype=np.float32)[:, csl] * scale).astype(f16)
        wk_c = np.asarray(Wk, dtype=np.float32)[:, csl].astype(f16)
        wv_c = np.asarray(Wv, dtype=np.float32)[:, csl].astype(f16)
        in_maps.append(
            {
                "xT": xT,
                "wq": np.ascontiguousarray(wq_c.reshape(KC, P, P).transpose(1, 0, 2)),
                "wk": np.ascontiguousarray(wk_c.reshape(KC, P, P).transpose(1, 0, 2)),
                "wv": np.ascontiguousarray(wv_c.reshape(KC, P, P).transpose(1, 0, 2)),
                "wo": np.ascontiguousarray(
                    np.asarray(Wo, dtype=np.float32)[csl, :].astype(f16)
                ),
                "bq": f32c(np.asarray(bq)[csl] * scale).reshape(P, 1),
                "bk": f32c(np.asarray(bk)[csl]).reshape(P, 1),
                "bv": f32c(np.asarray(bv)[csl]).reshape(P, 1),
                "maskT": maskT,
            }
        )

    trace = bool(os.environ.get("KERNEL_TRACE"))
    LAST_RESULTS = run_bass_kernel_spmd(
        nc, in_maps, list(range(NCORES)), trace=trace
    )

    acc = np.zeros((M, E), dtype=np.float64)
    for res in LAST_RESULTS.results:
        acc += res["out"].astype(np.float64)
    out = (acc + np.asarray(bo, dtype=np.float64)[None, :]).astype(np.float32)
    return out.reshape(B, S, E)


# revision 46
# speedup vs baseline: 1.2253x; 1.2253x over previous
"""Multi-head attention (B=2, S=2048, E=1024, H=16, D=64) on 8 TRN2 cores.

Sharding: tensor-parallel over heads. Core c owns heads {2c, 2c+1}:
  - Q/K/V projections column-sharded (128 cols each per core)
  - attention for the core's 2 heads (both batches)
  - out-projection row-sharded (128 rows of Wo) -> partial [4096,1024] f16
  - host sums the 8 partials and adds bo.

On-chip layout (everything "transposed"):
  - xT [1024, 4096] (E-major, fp16) is prefetched ONCE into SBUF in large
    contiguous chunks spread over four DMA queues (sync/scalar/vector/
    gpsimd), so no compute ever waits on an x load after ~2us
  - projections produce Q^T, K^T [128, 4096] (head-dim on partitions) and
    V^T, which is PE-transposed to token-major V tiles
  - scores are computed transposed: scores^T[kk, q] so softmax's key
    reduction can ride the attn@V matmul (ones-column in V) and the
    key-padding mask folds into the exp() per-partition bias
  - attn@V emits Y^T directly into a single stacked [128, M] tile
    (head 0 on partitions 0-63, head 1 on 64-127), so the out-projection
    is ONE K=128 matmul per tile instead of two K=64 ones
  - out partials are written f16 (halves the 16MB DRAM write), summed on
    host in float64.

Perf notes:
  - matmul inputs fp16 (full PE rate); accumulation fp32 in PSUM
  - matmul cost is streamed-columns only, so the schedule keeps the PE
    saturated and ACT (exp) just under it: exp() batched [128,1024],
    normalization on DVE+DMA (stride-0 partition-broadcast DMA), each
    pass's normalization emitted one pass late
  - deferred batch-1 projections: two m-chunks run right after phase 1,
    two are interleaved into attention passes 0-3 so the per-pass PE
    load matches the exp() pace instead of front-loading it
  - out-proj tiles interleave into passes 4-7; out writes ride the
    gpsimd DMA queue (idle engine -> free trigger slots)
"""

import os
import numpy as np

B, S, E, H, D = 2, 2048, 1024, 16, 64
M = B * S            # 4096 tokens
P = 128              # partitions
NCORES = 8
KC = E // P          # 8 contraction chunks for projections
MCH = 512            # token chunk for projections
QCH = 512            # query chunk for attention
NQC = S // QCH       # 4 query chunks per batch
NKT = S // P         # 16 key tiles per batch
NEG = -1.0e30

LAST_RESULTS = None  # BassKernelResults of the most recent run (for test harness)
_PROGRAM = None


def _build_program():
    import concourse.bass as bass
    import concourse.tile as tile
    from concourse import bacc, mybir
    from concourse.masks import make_identity

    f32 = mybir.dt.float32
    f16 = mybir.dt.float16

    nc = bacc.Bacc(
        "TRN2",
        target_bir_lowering=False,
        debug=False,
        enable_asserts=False,
        num_devices=NCORES,
    )

    xT_d = nc.dram_tensor("xT", (E, M), f16, kind="ExternalInput").ap()
    wq_d = nc.dram_tensor("wq", (P, KC, P), f16, kind="ExternalInput").ap()
    wk_d = nc.dram_tensor("wk", (P, KC, P), f16, kind="ExternalInput").ap()
    wv_d = nc.dram_tensor("wv", (P, KC, P), f16, kind="ExternalInput").ap()
    wo_d = nc.dram_tensor("wo", (P, E), f16, kind="ExternalInput").ap()
    bq_d = nc.dram_tensor("bq", (P, 1), f32, kind="ExternalInput").ap()
    bk_d = nc.dram_tensor("bk", (P, 1), f32, kind="ExternalInput").ap()
    bv_d = nc.dram_tensor("bv", (P, 1), f32, kind="ExternalInput").ap()
    maskT_d = nc.dram_tensor("maskT", (P, B * 16), f32, kind="ExternalInput").ap()
    out_d = nc.dram_tensor("out", (M, E), f16, kind="ExternalOutput").ap()
    rsc_d = nc.dram_tensor("rscratch", (16, QCH), f32, kind="Internal").ap()

    with tile.TileContext(nc) as tc:
        with (
            tc.tile_pool(name="consts", bufs=1) as consts,
            tc.tile_pool(name="big", bufs=1) as big,
            tc.tile_pool(name="vt_pool", bufs=2) as vt_pool,
            tc.tile_pool(name="pt_pool", bufs=8) as pt_pool,
            tc.tile_pool(name="r_pool", bufs=2) as r_pool,
            tc.tile_pool(name="out_pool", bufs=6) as out_pool,
        ):
            # ---- constants ----
            wq_sb = consts.tile([P, KC, P], f16)
            wk_sb = consts.tile([P, KC, P], f16)
            wv_sb = consts.tile([P, KC, P], f16)
            wo_sb = consts.tile([P, E], f16)
            bq_sb = consts.tile([P, 1], f32)
            bk_sb = consts.tile([P, 1], f32)
            bv_sb = consts.tile([P, 1], f32)
            mask_sb = consts.tile([P, B * 16], f32)
            ident = consts.tile([P, P], f16)
            ones_h = consts.tile([P, M // P], f16)

            # ---- resident x^T [128, KC, M]: 64KB/partition ----
            xsb = big.tile([P, KC, M], f16)

            # Prefetch: first weights + batch-0 x chunks round-robin over the
            # four DMA queues so nothing downstream waits on HBM.
            # CRITICAL: a dma_start on a backed-up DGE queue BLOCKS the
            # issuing engine's sequencer until a descriptor slot frees. The
            # 8MB x prefetch saturates its queues for ~30us, so x rides ONLY
            # sync+gpsimd (engines with no early work). ACT (scalar queue)
            # gets just the tiny consts, staying free for phase-1 staging.
            nc.sync.dma_start(wq_sb, wq_d)
            nc.scalar.dma_start(wk_sb, wk_d)
            nc.scalar.dma_start(wv_sb, wv_d)
            nc.scalar.dma_start(bq_sb, bq_d)
            nc.scalar.dma_start(bk_sb, bk_d)
            nc.scalar.dma_start(bv_sb, bv_d)
            nc.scalar.dma_start(mask_sb, maskT_d)
            nc.scalar.dma_start(wo_sb, wo_d)
            # gpsimd engine work must precede its x triggers (queue blocking)
            make_identity(nc, ident)
            nc.vector.memset(ones_h, 1.0)
            # x in m-chunk-sized pieces so phase-1 starts on the first piece
            # and the two queues stay ahead of the PE's kc-loop
            qs = [nc.sync, nc.gpsimd]
            di = 0
            for mc in range(M // MCH):
                msl = bass.ts(mc, MCH)
                for kc in range(KC):
                    qs[(di + 1) % 2].dma_start(
                        xsb[:, kc, msl], xT_d[bass.ts(kc, P), msl]
                    )
                    di += 1

            # ---- big persistent activations ----
            QT = big.tile([P, M], f16)       # Q^T: head-dims on partitions
            KT = big.tile([P, M], f16)
            # token-major V tiles: [tok, mt, 2*(64 cols + ones col)]
            Vtm = big.tile([P, M // P, 2 * (D + 1)], f16)
            YT = big.tile([P, M], f16)       # stacked attention output^T

            ones_col = ones_h[:, 0 : M // P].rearrange("p (a b) -> p a b", b=1)
            nc.vector.tensor_copy(Vtm[:, :, D : D + 1], ones_col)
            nc.vector.tensor_copy(Vtm[:, :, 2 * D + 1 : 2 * D + 2], ones_col)

            Exp = mybir.ActivationFunctionType.Exp
            Ident = mybir.ActivationFunctionType.Identity

            def emit_vt_tiles(mc, vt, psum_pool, tag):
                for j in range(MCH // P):
                    mt = mc * (MCH // P) + j
                    vtp = psum_pool.tile([P, P], f16, tag=tag, bufs=2, name=tag)
                    nc.tensor.transpose(vtp, vt[:, bass.ts(j, P)], ident)
                    nc.vector.tensor_copy(Vtm[:, mt, 0:D], vtp[:, 0:D])
                    nc.vector.tensor_copy(
                        Vtm[:, mt, D + 1 : 2 * D + 1], vtp[:, D : 2 * D]
                    )

            def make_proj_units(mc_pair, pool, tag, vtp_tag, stage_eng):
                # two m-chunks processed per weight load (the serialized
                # ldweights on a stationary switch is ~95ns; share it)
                units = []
                mcs = [(mc, bass.ts(mc, MCH)) for mc in mc_pair]
                state = {}

                def stage(dst, src, b_sb):
                    if stage_eng == "act":
                        nc.scalar.activation(dst, src, Ident, bias=b_sb)
                    else:
                        nc.vector.tensor_scalar_add(dst, src, b_sb)

                def u_proj(w_sb, which):
                    ps = [pool.tile([P, MCH], f32, tag=tag, name="pp2")
                          for _ in range(2)]
                    for kc in range(KC):
                        for i in range(2):
                            nc.tensor.matmul(
                                ps[i], w_sb[:, kc, :], xsb[:, kc, mcs[i][1]],
                                start=(kc == 0), stop=(kc == KC - 1),
                            )
                    state[which] = ps

                def u_q_mm():
                    u_proj(wq_sb, "q")

                def u_q_st():
                    for i in range(2):
                        stage(QT[:, mcs[i][1]], state["q"][i], bq_sb)

                def u_k_mm():
                    u_proj(wk_sb, "k")

                def u_k_st():
                    for i in range(2):
                        stage(KT[:, mcs[i][1]], state["k"][i], bk_sb)

                def u_v_mm():
                    u_proj(wv_sb, "v")

                def u_v_st():
                    vts = []
                    for i in range(2):
                        vt = vt_pool.tile([P, MCH], f16, name="vt2", tag="vt2")
                        stage(vt, state["v"][i], bv_sb)
                        vts.append(vt)
                    state["vts"] = vts

                def u_t0():
                    emit_vt_tiles(mc_pair[0], state["vts"][0], pool, vtp_tag)

                def u_t1():
                    emit_vt_tiles(mc_pair[1], state["vts"][1], pool, vtp_tag)

                units += [u_q_mm, u_q_st, u_k_mm, u_k_st,
                          u_v_mm, u_v_st, u_t0, u_t1]
                return units

            pre_fill = []   # deferred batch-0 V transposes (run in pass 0)
            holders = {}    # late-bound pool refs for deferred closures
            with (
                tc.tile_pool(name="psum_p1", bufs=6, space="PSUM") as psum_p1,
            ):
                # ---- phase 1: batch-0 projections, kc-outer within each
                # pair of m-chunks so compute starts on the first x chunk ----
                for grp in range(S // (2 * MCH)):
                    psums = []
                    for half in range(2):
                        mc = 2 * grp + half
                        msl = bass.ts(mc, MCH)
                        qp = psum_p1.tile([P, MCH], f32, tag="p1", name="qp")
                        kp = psum_p1.tile([P, MCH], f32, tag="p1", name="kp")
                        vp = psum_p1.tile([P, MCH], f32, tag="p1", name="vp")
                        psums.append((msl, qp, kp, vp))
                    for kc in range(KC):
                        st, sp = kc == 0, kc == KC - 1
                        for wi, w_sb in ((1, wq_sb), (2, wk_sb), (3, wv_sb)):
                            for half in range(2):
                                nc.tensor.matmul(
                                    psums[half][wi], w_sb[:, kc, :],
                                    xsb[:, kc, psums[half][0]],
                                    start=st, stop=sp,
                                )
                    for half in range(2):
                        msl, qp, kp, vp = psums[half]
                        mc = 2 * grp + half
                        # ACT is idle pre-attention: stage psum->sbuf there
                        nc.scalar.activation(QT[:, msl], qp, Ident, bias=bq_sb)
                        nc.scalar.activation(KT[:, msl], kp, Ident, bias=bk_sb)
                        vt = vt_pool.tile([P, MCH], f16, name="vt", bufs=4)
                        nc.scalar.activation(vt, vp, Ident, bias=bv_sb)
                        if mc == 0:
                            emit_vt_tiles(mc, vt, psum_p1, "vtp")
                        else:
                            # defer into pass-0 fillers: shaves ~5us off the
                            # serial head (only m-chunk-0 tiles are needed at
                            # pass-0 kt=0)
                            pre_fill.append(
                                lambda mc=mc, vt=vt: emit_vt_tiles(
                                    mc, vt, holders["op"], "op"
                                )
                            )

                # batch-1 projections are NOT run here: their x is still in
                # flight at phase-1 end; they interleave into passes 0-3

            # ---- phase 2: attention, deferred normalization, out-proj ----
            with (
                tc.tile_pool(name="psum_sc", bufs=2, space="PSUM") as psum_sc,
                tc.tile_pool(name="psum_av", bufs=2, space="PSUM") as psum_av,
                tc.tile_pool(name="psum_op", bufs=2, space="PSUM") as psum_op,
            ):
                if True:
                    norm_idx = [0]

                    def psum_to_sbuf(dst, src):
                        # DVE only: ACT must stay a pure-exp stream during
                        # attention or its stalls starve the PE
                        nc.vector.tensor_copy(dst, src)

                    def norm_dma(qi, av_sb):
                        # partition-broadcast the raw sums [1,512] -> [64,512]
                        # via DRAM bounce (SBUF-source DMAs cannot have a zero
                        # partition step)
                        qisl = bass.ts(qi, QCH)
                        ni = norm_idx[0]
                        norm_idx[0] += 1
                        nc.sync.dma_start(rsc_d[ni, :], av_sb[D : D + 1, qisl])
                        sb = r_pool.tile([D, QCH], f32, tag="sb", bufs=4, name="sb")
                        src = rsc_d[ni : ni + 1, :]
                        src_b = bass.AP(
                            tensor=src.tensor,
                            offset=src.offset,
                            ap=[[0, D]] + [list(x) for x in src.ap[1:]],
                        )
                        nc.sync.dma_start(sb, src_b)
                        return sb

                    def norm_dve(b, h, pr, qi, av_sb, sb):
                        # reciprocal at base partition 0 (custom-DVE approx
                        # ops misbehave at base 64), then scale into YT
                        qc = 2 * pr + qi
                        qsl = bass.ds(b * S + qc * QCH, QCH)
                        qisl = bass.ts(qi, QCH)
                        rbs = r_pool.tile([D, QCH], f32, tag="rbs", bufs=4, name="rbs")
                        # fast approx reciprocal: ~1e-3 rel err, well inside
                        # the tolerance, and 1 DVE instr instead of 3
                        nc.vector.reciprocal_approx_fast(rbs, sb)
                        nc.vector.tensor_mul(
                            YT[D * h : D * (h + 1), qsl], av_sb[0:D, qisl], rbs
                        )

                    def emit_norm_qc(b, h, pr, qi, av_sb):
                        sb = norm_dma(qi, av_sb)
                        norm_dve(b, h, pr, qi, av_sb, sb)

                    def emit_norm(b, h, pr, av_sb):
                        for qi in range(2):
                            emit_norm_qc(b, h, pr, qi, av_sb)

                    tail_mode = [False]

                    def emit_outproj_tile(b, j, spare_psum=False):
                        m0 = b * S + j * P
                        for ec in range(E // 512):
                            esl = bass.ts(ec, 512)
                            if spare_psum and ec % 2 == 1:
                                # after the last pass the score banks are free
                                op = psum_sc.tile([P, 512], f32, tag="sc", name="op_s")
                            else:
                                op = psum_op.tile([P, 512], f32, tag="op", name="op")
                            nc.tensor.matmul(
                                op, YT[:, bass.ds(m0, P)], wo_sb[:, esl],
                                start=True, stop=True,
                            )
                            osb = out_pool.tile([P, 512], f16, name="osb")
                            if (spare_psum or tail_mode[0]) and ec % 2 == 1:
                                # split tail staging across ACT and DVE so
                                # neither serializes the drain
                                nc.scalar.copy(osb, op)
                            else:
                                psum_to_sbuf(osb, op)
                            # tail: sync queue is reserved for norm bounces
                            oq = nc.gpsimd if (ec == 0 or tail_mode[0]) else nc.sync
                            oq.dma_start(out_d[bass.ds(m0, P), esl], osb)

                    def emit_outproj(b, jlo, jhi, spare_psum=False):
                        for j in range(jlo, jhi):
                            emit_outproj_tile(b, j, spare_psum)

                    passes = [
                        (b, h, pr) for b in range(B) for h in range(2) for pr in range(2)
                    ]
                    holders["op"] = psum_op
                    pending = []
                    filler = pre_fill
                    filler += list(make_proj_units([4, 5], psum_op, "op", "op", "dve"))
                    filler += list(make_proj_units([6, 7], psum_op, "op", "op", "dve"))
                    for pi, (b, h, pr) in enumerate(passes):
                        if pi == 4:
                            # deferred projections must be fully emitted before
                            # any batch-1 read (emission order defines dataflow)
                            while filler:
                                filler.pop(0)()
                            filler = [
                                (lambda b0=0, j0=j: emit_outproj_tile(b0, j0))
                                for j in range(S // P)
                            ]
                        dsl = bass.ds(D * h, D)
                        av2 = psum_av.tile(
                            [D + 1, 2 * QCH], f32, tag="av", bufs=1, name="av"
                        )

                        def av_pair(t, pt):
                            lhs_v = Vtm[:, b * 16 + t, h * (D + 1) : (h + 1) * (D + 1)]
                            for qi in range(2):
                                nc.tensor.matmul(
                                    av2[:, bass.ts(qi, QCH)], lhs_v,
                                    pt[:, bass.ts(qi, QCH)],
                                    start=(t == 0), stop=(t == NKT - 1),
                                )

                        # software-pipelined one kt deep: scores(t) and exp(t)
                        # are issued before av(t-1), so the PE's av never waits
                        # on the exp it consumes
                        prev_pt = None
                        for t in range(NKT):
                            ksl = bass.ds(b * S + t * P, P)
                            lhs_k = KT[dsl, ksl]
                            sc2 = psum_sc.tile([P, 2 * QCH], f32, tag="sc", name="sc2")
                            for qi in range(2):
                                qc = 2 * pr + qi
                                qsl = bass.ds(b * S + qc * QCH, QCH)
                                nc.tensor.matmul(
                                    sc2[:, bass.ts(qi, QCH)], lhs_k, QT[dsl, qsl],
                                    start=True, stop=True,
                                )
                            pt = pt_pool.tile([P, 2 * QCH], f16, tag="pt", name="pt")
                            bt = b * 16 + t
                            nc.scalar.activation(
                                pt, sc2, Exp, bias=mask_sb[:, bt : bt + 1], scale=1.0
                            )
                            if prev_pt is not None:
                                av_pair(t - 1, prev_pt)
                            prev_pt = pt
                            # early-emit the previous pass's normalization
                            # (DVE/DMA only) so its reciprocal never gates PE
                            if t == 1 and pending:
                                emit_norm(*pending.pop(0))
                                if pi == 7:
                                    # batch-1 pair-0 columns are now normalized
                                    filler.extend(
                                        (lambda b1=1, j1=j: emit_outproj_tile(b1, j1))
                                        for j in range(S // (2 * P))
                                    )
                            # interleave independent PE work (deferred
                            # projections, ready out-proj tiles); cadence per
                            # pass keeps each pass near the exp() pace.
                            # pass 0 starts late (batch-1 x still landing);
                            # pass 7 keeps 4 tiles in reserve for the tail
                            # norm bubble
                            if filler:
                                if pi == 0:
                                    if t in (1, 5, 9, 11, 13, 15):
                                        filler.pop(0)()
                                elif pi < 4:
                                    if t in (3, 7, 11, 15):
                                        filler.pop(0)()
                                elif pi == 7:
                                    # keep the pair-0 tiles in reserve: they
                                    # fill the PE during the tail norm's DRAM
                                    # round trip
                                    if t == 5:
                                        filler.pop(0)()
                                elif pi == 6:
                                    if t in (2, 6, 10, 14):
                                        filler.pop(0)()
                                elif t % 3 == 2:
                                    filler.pop(0)()
                        av_pair(NKT - 1, prev_pt)
                        # stage accumulators to SBUF, freeing the PSUM banks
                        av_sb = r_pool.tile(
                            [D + 1, 2 * QCH], f32, tag="avsb", bufs=3, name="avsb"
                        )
                        if pi == 7:
                            # last pass: denominator row staged separately on
                            # ACT (idle now) so the tail norm's DRAM bounce
                            # launches ~1.5us earlier
                            nc.scalar.copy(av_sb[D : D + 1, :], av2[D : D + 1, :])
                            nc.vector.tensor_copy(av_sb[0:D, :], av2[0:D, :])
                        else:
                            psum_to_sbuf(av_sb, av2)
                        pending.append((b, h, pr, av_sb))
                    # tail: drain remaining filler; interleave the last norm
                    # per-query-chunk with the out-proj tiles it unblocks
                    # tail: the last pass's denominator rows bounce straight
                    # from PSUM (av2 still live) so the broadcast RTT starts
                    # ~1.5us earlier; the reserved out-proj tiles fill the PE
                    # while the RTT is in flight
                    # tail order matters: both norm DMA bounces launch first,
                    # then the reserved out-proj tiles fill the PE (and the
                    # in-order DVE queue) while the DRAM round-trip is in
                    # flight, then the norms' DVE work and the final tiles
                    b_l, h_l, pr_l, av_sb_l = pending.pop(0)   # (1,1,1)
                    tail_mode[0] = True
                    sb0 = norm_dma(0, av_sb_l)
                    sb1 = norm_dma(1, av_sb_l)
                    while filler:
                        filler.pop(0)()
                    norm_dve(b_l, h_l, pr_l, 0, av_sb_l, sb0)
                    emit_outproj(1, 2 * S // (4 * P), 3 * S // (4 * P), spare_psum=True)
                    norm_dve(b_l, h_l, pr_l, 1, av_sb_l, sb1)
                    emit_outproj(1, 3 * S // (4 * P), S // P, spare_psum=True)

    nc.compile()
    return nc


def kernel(x, mask, Wq, bq, Wk, bk, Wv, bv, Wo, bo):
    global LAST_RESULTS, _PROGRAM
    from concourse.bass_utils import run_bass_kernel_spmd

    if _PROGRAM is None:
        _PROGRAM = _build_program()
    nc = _PROGRAM

    f16 = np.float16
    x = np.asarray(x, dtype=np.float32)
    mask = np.asarray(mask)
    f32c = lambda a: np.ascontiguousarray(np.asarray(a, dtype=np.float32))

    xT = np.ascontiguousarray(x.reshape(M, E).T.astype(f16))     # [E, M]
    maskf = np.where(mask, np.float32(NEG), np.float32(0.0)).astype(np.float32)
    maskT = np.ascontiguousarray(
        maskf.reshape(B, 16, P).transpose(2, 0, 1).reshape(P, B * 16)
    )
    scale = np.float32(1.0 / np.sqrt(D))

    in_maps = []
    for c in range(NCORES):
        csl = slice(P * c, P * (c + 1))
        wq_c = (np.asarray(Wq, dtype=np.float32)[:, csl] * scale).astype(f16)
        wk_c = np.asarray(Wk, dtype=np.float32)[:, csl].astype(f16)
        wv_c = np.asarray(Wv, dtype=np.float32)[:, csl].astype(f16)
        in_maps.append(
            {
                "xT": xT,
                "wq": np.ascontiguousarray(wq_c.reshape(KC, P, P).transpose(1, 0, 2)),
                "wk": np.ascontiguousarray(wk_c.reshape(KC, P, P).transpose(1, 0, 2)),
                "wv": np.ascontiguousarray(wv_c.reshape(KC, P, P).transpose(1, 0, 2)),
                "wo": np.ascontiguousarray(
                    np.asarray(Wo, dtype=np.float32)[csl, :].astype(f16)
                ),
                "bq": f32c(np.asarray(bq)[csl] * scale).reshape(P, 1),
                "bk": f32c(np.asarray(bk)[csl]).reshape(P, 1),
                "bv": f32c(np.asarray(bv)[csl]).reshape(P, 1),
                "maskT": maskT,
            }
        )

    trace = bool(os.environ.get("KERNEL_TRACE"))
    LAST_RESULTS = run_bass_kernel_spmd(
        nc, in_maps, list(range(NCORES)), trace=trace
    )

    acc = np.zeros((M, E), dtype=np.float64)
    for res in LAST_RESULTS.results:
        acc += res["out"].astype(np.float64)
    out = (acc + np.asarray(bo, dtype=np.float64)[None, :]).astype(np.float32)
    return out.reshape(B, S, E)


# revision 48
# speedup vs baseline: 1.2287x; 1.0028x over previous
"""Multi-head attention (B=2, S=2048, E=1024, H=16, D=64) on 8 TRN2 cores.

Sharding: tensor-parallel over heads. Core c owns heads {2c, 2c+1}:
  - Q/K/V projections column-sharded (128 cols each per core)
  - attention for the core's 2 heads (both batches)
  - out-projection row-sharded (128 rows of Wo) -> partial [4096,1024] f16
  - host sums the 8 partials and adds bo.

On-chip layout (everything "transposed"):
  - xT [1024, 4096] (E-major, fp16) is prefetched ONCE into SBUF in large
    contiguous chunks spread over four DMA queues (sync/scalar/vector/
    gpsimd), so no compute ever waits on an x load after ~2us
  - projections produce Q^T, K^T [128, 4096] (head-dim on partitions) and
    V^T, which is PE-transposed to token-major V tiles
  - scores are computed transposed: scores^T[kk, q] so softmax's key
    reduction can ride the attn@V matmul (ones-column in V) and the
    key-padding mask folds into the exp() per-partition bias
  - attn@V emits Y^T directly into a single stacked [128, M] tile
    (head 0 on partitions 0-63, head 1 on 64-127), so the out-projection
    is ONE K=128 matmul per tile instead of two K=64 ones
  - out partials are written f16 (halves the 16MB DRAM write), summed on
    host in float64.

Perf notes:
  - matmul inputs fp16 (full PE rate); accumulation fp32 in PSUM
  - matmul cost is streamed-columns only, so the schedule keeps the PE
    saturated and ACT (exp) just under it: exp() batched [128,1024],
    normalization on DVE+DMA (stride-0 partition-broadcast DMA), each
    pass's normalization emitted one pass late
  - deferred batch-1 projections: two m-chunks run right after phase 1,
    two are interleaved into attention passes 0-3 so the per-pass PE
    load matches the exp() pace instead of front-loading it
  - out-proj tiles interleave into passes 4-7; out writes ride the
    gpsimd DMA queue (idle engine -> free trigger slots)
"""

import os
import numpy as np

B, S, E, H, D = 2, 2048, 1024, 16, 64
M = B * S            # 4096 tokens
P = 128              # partitions
NCORES = 8
KC = E // P          # 8 contraction chunks for projections
MCH = 512            # token chunk for projections
QCH = 512            # query chunk for attention
NQC = S // QCH       # 4 query chunks per batch
NKT = S // P         # 16 key tiles per batch
NEG = -1.0e30

LAST_RESULTS = None  # BassKernelResults of the most recent run (for test harness)
_PROGRAM = None


def _build_program():
    import concourse.bass as bass
    import concourse.tile as tile
    from concourse import bacc, mybir
    from concourse.masks import make_identity

    f32 = mybir.dt.float32
    f16 = mybir.dt.float16

    nc = bacc.Bacc(
        "TRN2",
        target_bir_lowering=False,
        debug=False,
        enable_asserts=False,
        num_devices=NCORES,
    )

    xT_d = nc.dram_tensor("xT", (E, M), f16, kind="ExternalInput").ap()
    wq_d = nc.dram_tensor("wq", (P, KC, P), f16, kind="ExternalInput").ap()
    wk_d = nc.dram_tensor("wk", (P, KC, P), f16, kind="ExternalInput").ap()
    wv_d = nc.dram_tensor("wv", (P, KC, P), f16, kind="ExternalInput").ap()
    wo_d = nc.dram_tensor("wo", (P, E), f16, kind="ExternalInput").ap()
    bq_d = nc.dram_tensor("bq", (P, 1), f32, kind="ExternalInput").ap()
    bk_d = nc.dram_tensor("bk", (P, 1), f32, kind="ExternalInput").ap()
    bv_d = nc.dram_tensor("bv", (P, 1), f32, kind="ExternalInput").ap()
    maskT_d = nc.dram_tensor("maskT", (P, B * 16), f32, kind="ExternalInput").ap()
    out_d = nc.dram_tensor("out", (M, E), f16, kind="ExternalOutput").ap()
    rsc_d = nc.dram_tensor("rscratch", (16, QCH), f32, kind="Internal").ap()

    with tile.TileContext(nc) as tc:
        with (
            tc.tile_pool(name="consts", bufs=1) as consts,
            tc.tile_pool(name="big", bufs=1) as big,
            tc.tile_pool(name="vt_pool", bufs=2) as vt_pool,
            tc.tile_pool(name="pt_pool", bufs=8) as pt_pool,
            tc.tile_pool(name="r_pool", bufs=2) as r_pool,
            tc.tile_pool(name="out_pool", bufs=6) as out_pool,
        ):
            # ---- constants ----
            wq_sb = consts.tile([P, KC, P], f16)
            wk_sb = consts.tile([P, KC, P], f16)
            wv_sb = consts.tile([P, KC, P], f16)
            wo_sb = consts.tile([P, E], f16)
            bq_sb = consts.tile([P, 1], f32)
            bk_sb = consts.tile([P, 1], f32)
            bv_sb = consts.tile([P, 1], f32)
            mask_sb = consts.tile([P, B * 16], f32)
            ident = consts.tile([P, P], f16)
            ones_h = consts.tile([P, M // P], f16)

            # ---- resident x^T [128, KC, M]: 64KB/partition ----
            xsb = big.tile([P, KC, M], f16)

            # Prefetch: first weights + batch-0 x chunks round-robin over the
            # four DMA queues so nothing downstream waits on HBM.
            # CRITICAL: a dma_start on a backed-up DGE queue BLOCKS the
            # issuing engine's sequencer until a descriptor slot frees. The
            # 8MB x prefetch saturates its queues for ~30us, so x rides ONLY
            # sync+gpsimd (engines with no early work). ACT (scalar queue)
            # gets just the tiny consts, staying free for phase-1 staging.
            nc.sync.dma_start(wq_sb, wq_d)
            nc.scalar.dma_start(wk_sb, wk_d)
            nc.scalar.dma_start(wv_sb, wv_d)
            nc.scalar.dma_start(bq_sb, bq_d)
            nc.scalar.dma_start(bk_sb, bk_d)
            nc.scalar.dma_start(bv_sb, bv_d)
            nc.scalar.dma_start(mask_sb, maskT_d)
            nc.scalar.dma_start(wo_sb, wo_d)
            # gpsimd engine work must precede its x triggers (queue blocking)
            make_identity(nc, ident)
            nc.vector.memset(ones_h, 1.0)
            # x in m-chunk-sized pieces so phase-1 starts on the first piece
            # and the two queues stay ahead of the PE's kc-loop
            qs = [nc.sync, nc.gpsimd]
            di = 0
            for mc in range(M // MCH):
                msl = bass.ts(mc, MCH)
                for kc in range(KC):
                    qs[(di + 1) % 2].dma_start(
                        xsb[:, kc, msl], xT_d[bass.ts(kc, P), msl]
                    )
                    di += 1

            # ---- big persistent activations ----
            QT = big.tile([P, M], f16)       # Q^T: head-dims on partitions
            KT = big.tile([P, M], f16)
            # token-major V tiles: [tok, mt, 2*(64 cols + ones col)]
            Vtm = big.tile([P, M // P, 2 * (D + 1)], f16)
            YT = big.tile([P, M], f16)       # stacked attention output^T

            ones_col = ones_h[:, 0 : M // P].rearrange("p (a b) -> p a b", b=1)
            nc.vector.tensor_copy(Vtm[:, :, D : D + 1], ones_col)
            nc.vector.tensor_copy(Vtm[:, :, 2 * D + 1 : 2 * D + 2], ones_col)

            Exp = mybir.ActivationFunctionType.Exp
            Ident = mybir.ActivationFunctionType.Identity

            def emit_vt_tiles(mc, vt, psum_pool, tag):
                for j in range(MCH // P):
                    mt = mc * (MCH // P) + j
                    vtp = psum_pool.tile([P, P], f16, tag=tag, bufs=2, name=tag)
                    nc.tensor.transpose(vtp, vt[:, bass.ts(j, P)], ident)
                    nc.vector.tensor_copy(Vtm[:, mt, 0:D], vtp[:, 0:D])
                    nc.vector.tensor_copy(
                        Vtm[:, mt, D + 1 : 2 * D + 1], vtp[:, D : 2 * D]
                    )

            def make_proj_units(mc_pair, pool, tag, vtp_tag, stage_eng):
                # two m-chunks processed per weight load (the serialized
                # ldweights on a stationary switch is ~95ns; share it)
                units = []
                mcs = [(mc, bass.ts(mc, MCH)) for mc in mc_pair]
                state = {}

                def stage(dst, src, b_sb):
                    if stage_eng == "act":
                        nc.scalar.activation(dst, src, Ident, bias=b_sb)
                    else:
                        nc.vector.tensor_scalar_add(dst, src, b_sb)

                def u_proj(w_sb, which):
                    ps = [pool.tile([P, MCH], f32, tag=tag, name="pp2")
                          for _ in range(2)]
                    for kc in range(KC):
                        for i in range(2):
                            nc.tensor.matmul(
                                ps[i], w_sb[:, kc, :], xsb[:, kc, mcs[i][1]],
                                start=(kc == 0), stop=(kc == KC - 1),
                            )
                    state[which] = ps

                def u_q_mm():
                    u_proj(wq_sb, "q")

                def u_q_st():
                    for i in range(2):
                        stage(QT[:, mcs[i][1]], state["q"][i], bq_sb)

                def u_k_mm():
                    u_proj(wk_sb, "k")

                def u_k_st():
                    for i in range(2):
                        stage(KT[:, mcs[i][1]], state["k"][i], bk_sb)

                def u_v_mm():
                    u_proj(wv_sb, "v")

                def u_v_st():
                    vts = []
                    for i in range(2):
                        vt = vt_pool.tile([P, MCH], f16, name="vt2", tag="vt2")
                        stage(vt, state["v"][i], bv_sb)
                        vts.append(vt)
                    state["vts"] = vts

                def u_t0():
                    emit_vt_tiles(mc_pair[0], state["vts"][0], pool, vtp_tag)

                def u_t1():
                    emit_vt_tiles(mc_pair[1], state["vts"][1], pool, vtp_tag)

                units += [u_q_mm, u_q_st, u_k_mm, u_k_st,
                          u_v_mm, u_v_st, u_t0, u_t1]
                return units

            pre_fill = []   # deferred batch-0 V transposes (run in pass 0)
            holders = {}    # late-bound pool refs for deferred closures
            with (
                tc.tile_pool(name="psum_p1", bufs=6, space="PSUM") as psum_p1,
            ):
                # ---- phase 1: batch-0 projections, kc-outer within each
                # pair of m-chunks so compute starts on the first x chunk ----
                for grp in range(S // (2 * MCH)):
                    psums = []
                    for half in range(2):
                        mc = 2 * grp + half
                        msl = bass.ts(mc, MCH)
                        qp = psum_p1.tile([P, MCH], f32, tag="p1", name="qp")
                        kp = psum_p1.tile([P, MCH], f32, tag="p1", name="kp")
                        vp = psum_p1.tile([P, MCH], f32, tag="p1", name="vp")
                        psums.append((msl, qp, kp, vp))
                    for kc in range(KC):
                        st, sp = kc == 0, kc == KC - 1
                        for wi, w_sb in ((1, wq_sb), (2, wk_sb), (3, wv_sb)):
                            for half in range(2):
                                nc.tensor.matmul(
                                    psums[half][wi], w_sb[:, kc, :],
                                    xsb[:, kc, psums[half][0]],
                                    start=st, stop=sp,
                                )
                    for half in range(2):
                        msl, qp, kp, vp = psums[half]
                        mc = 2 * grp + half
                        # ACT is idle pre-attention: stage psum->sbuf there
                        nc.scalar.activation(QT[:, msl], qp, Ident, bias=bq_sb)
                        nc.scalar.activation(KT[:, msl], kp, Ident, bias=bk_sb)
                        vt = vt_pool.tile([P, MCH], f16, name="vt", bufs=4)
                        nc.scalar.activation(vt, vp, Ident, bias=bv_sb)
                        emit_vt_tiles(mc, vt, psum_p1, "vtp")

                # batch-1 projections are NOT run here: their x is still in
                # flight at phase-1 end; they interleave into passes 0-3

            # ---- phase 2: attention, deferred normalization, out-proj ----
            with (
                tc.tile_pool(name="psum_sc", bufs=2, space="PSUM") as psum_sc,
                tc.tile_pool(name="psum_av", bufs=2, space="PSUM") as psum_av,
                tc.tile_pool(name="psum_op", bufs=2, space="PSUM") as psum_op,
            ):
                if True:
                    norm_idx = [0]

                    def psum_to_sbuf(dst, src):
                        # DVE only: ACT must stay a pure-exp stream during
                        # attention or its stalls starve the PE
                        nc.vector.tensor_copy(dst, src)

                    def norm_dma(qi, av_sb):
                        # partition-broadcast the raw sums [1,512] -> [64,512]
                        # via DRAM bounce (SBUF-source DMAs cannot have a zero
                        # partition step)
                        qisl = bass.ts(qi, QCH)
                        ni = norm_idx[0]
                        norm_idx[0] += 1
                        nc.sync.dma_start(rsc_d[ni, :], av_sb[D : D + 1, qisl])
                        sb = r_pool.tile([D, QCH], f32, tag="sb", bufs=4, name="sb")
                        src = rsc_d[ni : ni + 1, :]
                        src_b = bass.AP(
                            tensor=src.tensor,
                            offset=src.offset,
                            ap=[[0, D]] + [list(x) for x in src.ap[1:]],
                        )
                        nc.sync.dma_start(sb, src_b)
                        return sb

                    def norm_dve(b, h, pr, qi, av_sb, sb):
                        # reciprocal at base partition 0 (custom-DVE approx
                        # ops misbehave at base 64), then scale into YT
                        qc = 2 * pr + qi
                        qsl = bass.ds(b * S + qc * QCH, QCH)
                        qisl = bass.ts(qi, QCH)
                        rbs = r_pool.tile([D, QCH], f32, tag="rbs", bufs=4, name="rbs")
                        # fast approx reciprocal: ~1e-3 rel err, well inside
                        # the tolerance, and 1 DVE instr instead of 3
                        nc.vector.reciprocal_approx_fast(rbs, sb)
                        nc.vector.tensor_mul(
                            YT[D * h : D * (h + 1), qsl], av_sb[0:D, qisl], rbs
                        )

                    def emit_norm_qc(b, h, pr, qi, av_sb):
                        sb = norm_dma(qi, av_sb)
                        norm_dve(b, h, pr, qi, av_sb, sb)

                    def emit_norm(b, h, pr, av_sb):
                        for qi in range(2):
                            emit_norm_qc(b, h, pr, qi, av_sb)

                    tail_mode = [False]

                    def emit_outproj_tile(b, j, spare_psum=False):
                        m0 = b * S + j * P
                        for ec in range(E // 512):
                            esl = bass.ts(ec, 512)
                            if spare_psum and ec % 2 == 1:
                                # after the last pass the score banks are free
                                op = psum_sc.tile([P, 512], f32, tag="sc", name="op_s")
                            else:
                                op = psum_op.tile([P, 512], f32, tag="op", name="op")
                            nc.tensor.matmul(
                                op, YT[:, bass.ds(m0, P)], wo_sb[:, esl],
                                start=True, stop=True,
                            )
                            osb = out_pool.tile([P, 512], f16, name="osb")
                            if (spare_psum or tail_mode[0]) and ec % 2 == 1:
                                # split tail staging across ACT and DVE so
                                # neither serializes the drain
                                nc.scalar.copy(osb, op)
                            else:
                                psum_to_sbuf(osb, op)
                            # tail: sync queue is reserved for norm bounces
                            oq = nc.gpsimd if (ec == 0 or tail_mode[0]) else nc.sync
                            oq.dma_start(out_d[bass.ds(m0, P), esl], osb)

                    def emit_outproj(b, jlo, jhi, spare_psum=False):
                        for j in range(jlo, jhi):
                            emit_outproj_tile(b, j, spare_psum)

                    passes = [
                        (b, h, pr) for b in range(B) for h in range(2) for pr in range(2)
                    ]
                    holders["op"] = psum_op
                    pending = []
                    filler = pre_fill
                    filler += list(make_proj_units([4, 5], psum_op, "op", "op", "dve"))
                    filler += list(make_proj_units([6, 7], psum_op, "op", "op", "dve"))
                    for pi, (b, h, pr) in enumerate(passes):
                        if pi == 4:
                            # deferred projections must be fully emitted before
                            # any batch-1 read (emission order defines dataflow)
                            while filler:
                                filler.pop(0)()
                            filler = [
                                (lambda b0=0, j0=j: emit_outproj_tile(b0, j0))
                                for j in range(S // P)
                            ]
                        dsl = bass.ds(D * h, D)
                        av2 = psum_av.tile(
                            [D + 1, 2 * QCH], f32, tag="av", bufs=1, name="av"
                        )

                        def av_pair(t, pt):
                            lhs_v = Vtm[:, b * 16 + t, h * (D + 1) : (h + 1) * (D + 1)]
                            for qi in range(2):
                                nc.tensor.matmul(
                                    av2[:, bass.ts(qi, QCH)], lhs_v,
                                    pt[:, bass.ts(qi, QCH)],
                                    start=(t == 0), stop=(t == NKT - 1),
                                )

                        # software-pipelined one kt deep: scores(t) and exp(t)
                        # are issued before av(t-1), so the PE's av never waits
                        # on the exp it consumes
                        prev_pt = None
                        for t in range(NKT):
                            ksl = bass.ds(b * S + t * P, P)
                            lhs_k = KT[dsl, ksl]
                            sc2 = psum_sc.tile([P, 2 * QCH], f32, tag="sc", name="sc2")
                            for qi in range(2):
                                qc = 2 * pr + qi
                                qsl = bass.ds(b * S + qc * QCH, QCH)
                                nc.tensor.matmul(
                                    sc2[:, bass.ts(qi, QCH)], lhs_k, QT[dsl, qsl],
                                    start=True, stop=True,
                                )
                            pt = pt_pool.tile([P, 2 * QCH], f16, tag="pt", name="pt")
                            bt = b * 16 + t
                            nc.scalar.activation(
                                pt, sc2, Exp, bias=mask_sb[:, bt : bt + 1], scale=1.0
                            )
                            if prev_pt is not None:
                                av_pair(t - 1, prev_pt)
                            prev_pt = pt
                            # early-emit the previous pass's normalization
                            # (DVE/DMA only) so its reciprocal never gates PE
                            if t == 1 and pending:
                                emit_norm(*pending.pop(0))
                                if pi == 7:
                                    # batch-1 pair-0 columns are now normalized
                                    filler.extend(
                                        (lambda b1=1, j1=j: emit_outproj_tile(b1, j1))
                                        for j in range(S // (2 * P))
                                    )
                            # interleave independent PE work (deferred
                            # projections, ready out-proj tiles); cadence per
                            # pass keeps each pass near the exp() pace.
                            # pass 0 starts late (batch-1 x still landing);
                            # pass 7 keeps 4 tiles in reserve for the tail
                            # norm bubble
                            if filler:
                                if pi == 0:
                                    if t in (7, 10, 13, 15):
                                        filler.pop(0)()
                                elif pi < 4:
                                    if t in (3, 7, 11, 15):
                                        filler.pop(0)()
                                elif pi == 7:
                                    # keep the pair-0 tiles in reserve: they
                                    # fill the PE during the tail norm's DRAM
                                    # round trip
                                    if t == 5:
                                        filler.pop(0)()
                                elif pi == 6:
                                    if t in (2, 6, 10, 14):
                                        filler.pop(0)()
                                elif t % 3 == 2:
                                    filler.pop(0)()
                        av_pair(NKT - 1, prev_pt)
                        # stage accumulators to SBUF, freeing the PSUM banks
                        av_sb = r_pool.tile(
                            [D + 1, 2 * QCH], f32, tag="avsb", bufs=3, name="avsb"
                        )
                        if pi == 7:
                            # last pass: denominator row staged separately on
                            # ACT (idle now) so the tail norm's DRAM bounce
                            # launches ~1.5us earlier
                            nc.scalar.copy(av_sb[D : D + 1, :], av2[D : D + 1, :])
                            nc.vector.tensor_copy(av_sb[0:D, :], av2[0:D, :])
                        else:
                            psum_to_sbuf(av_sb, av2)
                        pending.append((b, h, pr, av_sb))
                    # tail: drain remaining filler; interleave the last norm
                    # per-query-chunk with the out-proj tiles it unblocks
                    # tail: the last pass's denominator rows bounce straight
                    # from PSUM (av2 still live) so the broadcast RTT starts
                    # ~1.5us earlier; the reserved out-proj tiles fill the PE
                    # while the RTT is in flight
                    # tail order matters: both norm DMA bounces launch first,
                    # then the reserved out-proj tiles fill the PE (and the
                    # in-order DVE queue) while the DRAM round-trip is in
                    # flight, then the norms' DVE work and the final tiles
                    b_l, h_l, pr_l, av_sb_l = pending.pop(0)   # (1,1,1)
                    tail_mode[0] = True
                    sb0 = norm_dma(0, av_sb_l)
                    sb1 = norm_dma(1, av_sb_l)
                    while filler:
                        filler.pop(0)()
                    norm_dve(b_l, h_l, pr_l, 0, av_sb_l, sb0)
                    emit_outproj(1, 2 * S // (4 * P), 3 * S // (4 * P), spare_psum=True)
                    norm_dve(b_l, h_l, pr_l, 1, av_sb_l, sb1)
                    emit_outproj(1, 3 * S // (4 * P), S // P, spare_psum=True)

    nc.compile()
    return nc


def kernel(x, mask, Wq, bq, Wk, bk, Wv, bv, Wo, bo):
    global LAST_RESULTS, _PROGRAM
    from concourse.bass_utils import run_bass_kernel_spmd

    if _PROGRAM is None:
        _PROGRAM = _build_program()
    nc = _PROGRAM

    f16 = np.float16
    x = np.asarray(x, dtype=np.float32)
    mask = np.asarray(mask)
    f32c = lambda a: np.ascontiguousarray(np.asarray(a, dtype=np.float32))

    xT = np.ascontiguousarray(x.reshape(M, E).T.astype(f16))     # [E, M]
    maskf = np.where(mask, np.float32(NEG), np.float32(0.0)).astype(np.float32)
    maskT = np.ascontiguousarray(
        maskf.reshape(B, 16, P).transpose(2, 0, 1).reshape(P, B * 16)
    )
    scale = np.float32(1.0 / np.sqrt(D))

    in_maps = []
    for c in range(NCORES):
        csl = slice(P * c, P * (c + 1))
        wq_c = (np.asarray(Wq, dtype=np.float32)[:, csl] * scale).astype(f16)
        wk_c = np.asarray(Wk, dtype=np.float32)[:, csl].astype(f16)
        wv_c = np.asarray(Wv, dtype=np.float32)[:, csl].astype(f16)
        in_maps.append(
            {
                "xT": xT,
                "wq": np.ascontiguousarray(wq_c.reshape(KC, P, P).transpose(1, 0, 2)),
                "wk": np.ascontiguousarray(wk_c.reshape(KC, P, P).transpose(1, 0, 2)),
                "wv": np.ascontiguousarray(wv_c.reshape(KC, P, P).transpose(1, 0, 2)),
                "wo": np.ascontiguousarray(
                    np.asarray(Wo, dtype=np.float32)[csl, :].astype(f16)
                ),
                "bq": f32c(np.asarray(bq)[csl] * scale).reshape(P, 1),
                "bk": f32c(np.asarray(bk)[csl]).reshape(P, 1),
                "bv": f32c(np.asarray(bv)[csl]).reshape(P, 1),
                "maskT": maskT,
            }
        )

    trace = bool(os.environ.get("KERNEL_TRACE"))
    LAST_RESULTS = run_bass_kernel_spmd(
        nc, in_maps, list(range(NCORES)), trace=trace
    )

    acc = np.zeros((M, E), dtype=np.float64)
    for res in LAST_RESULTS.results:
        acc += res["out"].astype(np.float64)
    out = (acc + np.asarray(bo, dtype=np.float64)[None, :]).astype(np.float32)
    return out.reshape(B, S, E)


# revision 49
# speedup vs baseline: 1.2446x; 1.0130x over previous
"""Multi-head attention (B=2, S=2048, E=1024, H=16, D=64) on 8 TRN2 cores.

Sharding: tensor-parallel over heads. Core c owns heads {2c, 2c+1}:
  - Q/K/V projections column-sharded (128 cols each per core)
  - attention for the core's 2 heads (both batches)
  - out-projection row-sharded (128 rows of Wo) -> partial [4096,1024] f16
  - host sums the 8 partials and adds bo.

On-chip layout (everything "transposed"):
  - xT [1024, 4096] (E-major, fp16) is prefetched ONCE into SBUF in large
    contiguous chunks spread over four DMA queues (sync/scalar/vector/
    gpsimd), so no compute ever waits on an x load after ~2us
  - projections produce Q^T, K^T [128, 4096] (head-dim on partitions) and
    V^T, which is PE-transposed to token-major V tiles
  - scores are computed transposed: scores^T[kk, q] so softmax's key
    reduction can ride the attn@V matmul (ones-column in V) and the
    key-padding mask folds into the exp() per-partition bias
  - attn@V emits Y^T directly into a single stacked [128, M] tile
    (head 0 on partitions 0-63, head 1 on 64-127), so the out-projection
    is ONE K=128 matmul per tile instead of two K=64 ones
  - out partials are written f16 (halves the 16MB DRAM write), summed on
    host in float64.

Perf notes:
  - matmul inputs fp16 (full PE rate); accumulation fp32 in PSUM
  - matmul cost is streamed-columns only, so the schedule keeps the PE
    saturated and ACT (exp) just under it: exp() batched [128,1024],
    normalization on DVE+DMA (stride-0 partition-broadcast DMA), each
    pass's normalization emitted one pass late
  - deferred batch-1 projections: two m-chunks run right after phase 1,
    two are interleaved into attention passes 0-3 so the per-pass PE
    load matches the exp() pace instead of front-loading it
  - out-proj tiles interleave into passes 4-7; out writes ride the
    gpsimd DMA queue (idle engine -> free trigger slots)
"""

import os
import numpy as np

B, S, E, H, D = 2, 2048, 1024, 16, 64
M = B * S            # 4096 tokens
P = 128              # partitions
NCORES = 8
KC = E // P          # 8 contraction chunks for projections
MCH = 512            # token chunk for projections
QCH = 512            # query chunk for attention
NQC = S // QCH       # 4 query chunks per batch
NKT = S // P         # 16 key tiles per batch
NEG = -1.0e30

LAST_RESULTS = None  # BassKernelResults of the most recent run (for test harness)
_PROGRAM = None


def _build_program():
    import concourse.bass as bass
    import concourse.tile as tile
    from concourse import bacc, mybir
    from concourse.masks import make_identity

    f32 = mybir.dt.float32
    f16 = mybir.dt.float16

    nc = bacc.Bacc(
        "TRN2",
        target_bir_lowering=False,
        debug=False,
        enable_asserts=False,
        num_devices=NCORES,
    )

    xT_d = nc.dram_tensor("xT", (E, M), f16, kind="ExternalInput").ap()
    wq_d = nc.dram_tensor("wq", (P, KC, P), f16, kind="ExternalInput").ap()
    wk_d = nc.dram_tensor("wk", (P, KC, P), f16, kind="ExternalInput").ap()
    wv_d = nc.dram_tensor("wv", (P, KC, P), f16, kind="ExternalInput").ap()
    wo_d = nc.dram_tensor("wo", (P, E), f16, kind="ExternalInput").ap()
    bq_d = nc.dram_tensor("bq", (P, 1), f32, kind="ExternalInput").ap()
    bk_d = nc.dram_tensor("bk", (P, 1), f32, kind="ExternalInput").ap()
    bv_d = nc.dram_tensor("bv", (P, 1), f32, kind="ExternalInput").ap()
    maskT_d = nc.dram_tensor("maskT", (P, B * 16), f32, kind="ExternalInput").ap()
    out_d = nc.dram_tensor("out", (M, E), f16, kind="ExternalOutput").ap()
    rsc_d = nc.dram_tensor("rscratch", (16, QCH), f32, kind="Internal").ap()

    with tile.TileContext(nc) as tc:
        with (
            tc.tile_pool(name="consts", bufs=1) as consts,
            tc.tile_pool(name="big", bufs=1) as big,
            tc.tile_pool(name="vt_pool", bufs=2) as vt_pool,
            tc.tile_pool(name="pt_pool", bufs=8) as pt_pool,
            tc.tile_pool(name="r_pool", bufs=2) as r_pool,
            tc.tile_pool(name="out_pool", bufs=6) as out_pool,
        ):
            # ---- constants ----
            wq_sb = consts.tile([P, KC, P], f16)
            wk_sb = consts.tile([P, KC, P], f16)
            wv_sb = consts.tile([P, KC, P], f16)
            wo_sb = consts.tile([P, E], f16)
            bq_sb = consts.tile([P, 1], f32)
            bk_sb = consts.tile([P, 1], f32)
            bv_sb = consts.tile([P, 1], f32)
            mask_sb = consts.tile([P, B * 16], f32)
            ident = consts.tile([P, P], f16)
            ones_h = consts.tile([P, M // P], f16)

            # ---- resident x^T [128, KC, M]: 64KB/partition ----
            xsb = big.tile([P, KC, M], f16)

            # Prefetch: first weights + batch-0 x chunks round-robin over the
            # four DMA queues so nothing downstream waits on HBM.
            # CRITICAL: a dma_start on a backed-up DGE queue BLOCKS the
            # issuing engine's sequencer until a descriptor slot frees. The
            # 8MB x prefetch saturates its queues for ~30us, so x rides ONLY
            # sync+gpsimd (engines with no early work). ACT (scalar queue)
            # gets just the tiny consts, staying free for phase-1 staging.
            nc.sync.dma_start(wq_sb, wq_d)
            nc.scalar.dma_start(wk_sb, wk_d)
            nc.scalar.dma_start(wv_sb, wv_d)
            nc.scalar.dma_start(bq_sb, bq_d)
            nc.scalar.dma_start(bk_sb, bk_d)
            nc.scalar.dma_start(bv_sb, bv_d)
            nc.scalar.dma_start(mask_sb, maskT_d)
            nc.scalar.dma_start(wo_sb, wo_d)
            # gpsimd engine work must precede its x triggers (queue blocking)
            make_identity(nc, ident)
            nc.vector.memset(ones_h, 1.0)
            # x in m-chunk-sized pieces so phase-1 starts on the first piece
            # and the two queues stay ahead of the PE's kc-loop
            qs = [nc.sync, nc.gpsimd]
            di = 0
            for mc in range(M // MCH):
                msl = bass.ts(mc, MCH)
                for kc in range(KC):
                    qs[(di + 1) % 2].dma_start(
                        xsb[:, kc, msl], xT_d[bass.ts(kc, P), msl]
                    )
                    di += 1

            # ---- big persistent activations ----
            QT = big.tile([P, M], f16)       # Q^T: head-dims on partitions
            KT = big.tile([P, M], f16)
            # token-major V tiles: [tok, mt, 2*(64 cols + ones col)]
            Vtm = big.tile([P, M // P, 2 * (D + 1)], f16)
            YT = big.tile([P, M], f16)       # stacked attention output^T

            ones_col = ones_h[:, 0 : M // P].rearrange("p (a b) -> p a b", b=1)
            nc.vector.tensor_copy(Vtm[:, :, D : D + 1], ones_col)
            nc.vector.tensor_copy(Vtm[:, :, 2 * D + 1 : 2 * D + 2], ones_col)

            Exp = mybir.ActivationFunctionType.Exp
            Ident = mybir.ActivationFunctionType.Identity

            def emit_vt_tiles(mc, vt, psum_pool, tag):
                for j in range(MCH // P):
                    mt = mc * (MCH // P) + j
                    vtp = psum_pool.tile([P, P], f16, tag=tag, bufs=2, name=tag)
                    nc.tensor.transpose(vtp, vt[:, bass.ts(j, P)], ident)
                    nc.vector.tensor_copy(Vtm[:, mt, 0:D], vtp[:, 0:D])
                    nc.vector.tensor_copy(
                        Vtm[:, mt, D + 1 : 2 * D + 1], vtp[:, D : 2 * D]
                    )

            def make_proj_units(mc_pair, pool, tag, vtp_tag, stage_eng):
                # two m-chunks processed per weight load (the serialized
                # ldweights on a stationary switch is ~95ns; share it)
                units = []
                mcs = [(mc, bass.ts(mc, MCH)) for mc in mc_pair]
                state = {}

                def stage(dst, src, b_sb):
                    if stage_eng == "act":
                        nc.scalar.activation(dst, src, Ident, bias=b_sb)
                    else:
                        nc.vector.tensor_scalar_add(dst, src, b_sb)

                def u_proj(w_sb, which):
                    ps = [pool.tile([P, MCH], f32, tag=tag, name="pp2")
                          for _ in range(2)]
                    for kc in range(KC):
                        for i in range(2):
                            nc.tensor.matmul(
                                ps[i], w_sb[:, kc, :], xsb[:, kc, mcs[i][1]],
                                start=(kc == 0), stop=(kc == KC - 1),
                            )
                    state[which] = ps

                def u_q_mm():
                    u_proj(wq_sb, "q")

                def u_q_st():
                    for i in range(2):
                        stage(QT[:, mcs[i][1]], state["q"][i], bq_sb)

                def u_k_mm():
                    u_proj(wk_sb, "k")

                def u_k_st():
                    for i in range(2):
                        stage(KT[:, mcs[i][1]], state["k"][i], bk_sb)

                def u_v_mm():
                    u_proj(wv_sb, "v")

                def u_v_st():
                    vts = []
                    for i in range(2):
                        vt = vt_pool.tile([P, MCH], f16, name="vt2", tag="vt2")
                        stage(vt, state["v"][i], bv_sb)
                        vts.append(vt)
                    state["vts"] = vts

                def u_t0():
                    emit_vt_tiles(mc_pair[0], state["vts"][0], pool, vtp_tag)

                def u_t1():
                    emit_vt_tiles(mc_pair[1], state["vts"][1], pool, vtp_tag)

                units += [u_q_mm, u_q_st, u_k_mm, u_k_st,
                          u_v_mm, u_v_st, u_t0, u_t1]
                return units

            pre_fill = []   # deferred batch-0 V transposes (run in pass 0)
            holders = {}    # late-bound pool refs for deferred closures
            with (
                tc.tile_pool(name="psum_p1", bufs=6, space="PSUM") as psum_p1,
            ):
                # ---- phase 1: batch-0 projections, kc-outer within each
                # pair of m-chunks so compute starts on the first x chunk ----
                for grp in range(S // (2 * MCH)):
                    psums = []
                    for half in range(2):
                        mc = 2 * grp + half
                        msl = bass.ts(mc, MCH)
                        qp = psum_p1.tile([P, MCH], f32, tag="p1", name="qp")
                        kp = psum_p1.tile([P, MCH], f32, tag="p1", name="kp")
                        vp = psum_p1.tile([P, MCH], f32, tag="p1", name="vp")
                        psums.append((msl, qp, kp, vp))
                    for kc in range(KC):
                        st, sp = kc == 0, kc == KC - 1
                        for wi, w_sb in ((1, wq_sb), (2, wk_sb), (3, wv_sb)):
                            for half in range(2):
                                nc.tensor.matmul(
                                    psums[half][wi], w_sb[:, kc, :],
                                    xsb[:, kc, psums[half][0]],
                                    start=st, stop=sp,
                                )
                    for half in range(2):
                        msl, qp, kp, vp = psums[half]
                        mc = 2 * grp + half
                        # ACT is idle pre-attention: stage psum->sbuf there
                        nc.scalar.activation(QT[:, msl], qp, Ident, bias=bq_sb)
                        nc.scalar.activation(KT[:, msl], kp, Ident, bias=bk_sb)
                        vt = vt_pool.tile([P, MCH], f16, name="vt", bufs=4)
                        nc.scalar.activation(vt, vp, Ident, bias=bv_sb)
                        emit_vt_tiles(mc, vt, psum_p1, "vtp")

                # batch-1 projections are NOT run here: their x is still in
                # flight at phase-1 end; they interleave into passes 0-3

            # ---- phase 2: attention, deferred normalization, out-proj ----
            with (
                tc.tile_pool(name="psum_sc", bufs=2, space="PSUM") as psum_sc,
                tc.tile_pool(name="psum_av", bufs=2, space="PSUM") as psum_av,
                tc.tile_pool(name="psum_op", bufs=2, space="PSUM") as psum_op,
            ):
                if True:
                    norm_idx = [0]

                    def psum_to_sbuf(dst, src):
                        # DVE only: ACT must stay a pure-exp stream during
                        # attention or its stalls starve the PE
                        nc.vector.tensor_copy(dst, src)

                    def norm_dma(qi, av_sb):
                        # partition-broadcast the raw sums [1,512] -> [64,512]
                        # via DRAM bounce (SBUF-source DMAs cannot have a zero
                        # partition step)
                        qisl = bass.ts(qi, QCH)
                        ni = norm_idx[0]
                        norm_idx[0] += 1
                        nc.sync.dma_start(rsc_d[ni, :], av_sb[D : D + 1, qisl])
                        sb = r_pool.tile([D, QCH], f32, tag="sb", bufs=4, name="sb")
                        src = rsc_d[ni : ni + 1, :]
                        src_b = bass.AP(
                            tensor=src.tensor,
                            offset=src.offset,
                            ap=[[0, D]] + [list(x) for x in src.ap[1:]],
                        )
                        nc.sync.dma_start(sb, src_b)
                        return sb

                    def norm_dve(b, h, pr, qi, av_sb, sb):
                        # reciprocal at base partition 0 (custom-DVE approx
                        # ops misbehave at base 64), then scale into YT
                        qc = 2 * pr + qi
                        qsl = bass.ds(b * S + qc * QCH, QCH)
                        qisl = bass.ts(qi, QCH)
                        rbs = r_pool.tile([D, QCH], f32, tag="rbs", bufs=4, name="rbs")
                        # fast approx reciprocal: ~1e-3 rel err, well inside
                        # the tolerance, and 1 DVE instr instead of 3
                        nc.vector.reciprocal_approx_fast(rbs, sb)
                        nc.vector.tensor_mul(
                            YT[D * h : D * (h + 1), qsl], av_sb[0:D, qisl], rbs
                        )

                    def emit_norm_qc(b, h, pr, qi, av_sb):
                        sb = norm_dma(qi, av_sb)
                        norm_dve(b, h, pr, qi, av_sb, sb)

                    def emit_norm(b, h, pr, av_sb):
                        for qi in range(2):
                            emit_norm_qc(b, h, pr, qi, av_sb)

                    tail_mode = [False]

                    def emit_outproj_tile(b, j, spare_psum=False):
                        m0 = b * S + j * P
                        for ec in range(E // 512):
                            esl = bass.ts(ec, 512)
                            if spare_psum and ec % 2 == 1:
                                # after the last pass the score banks are free
                                op = psum_sc.tile([P, 512], f32, tag="sc", name="op_s")
                            else:
                                op = psum_op.tile([P, 512], f32, tag="op", name="op")
                            nc.tensor.matmul(
                                op, YT[:, bass.ds(m0, P)], wo_sb[:, esl],
                                start=True, stop=True,
                            )
                            osb = out_pool.tile([P, 512], f16, name="osb")
                            if (spare_psum or tail_mode[0]) and ec % 2 == 1:
                                # split tail staging across ACT and DVE so
                                # neither serializes the drain
                                nc.scalar.copy(osb, op)
                            else:
                                psum_to_sbuf(osb, op)
                            # tail: sync queue is reserved for norm bounces
                            oq = nc.gpsimd if (ec == 0 or tail_mode[0]) else nc.sync
                            oq.dma_start(out_d[bass.ds(m0, P), esl], osb)

                    def emit_outproj(b, jlo, jhi, spare_psum=False):
                        for j in range(jlo, jhi):
                            emit_outproj_tile(b, j, spare_psum)

                    passes = [
                        (b, h, pr) for b in range(B) for h in range(2) for pr in range(2)
                    ]
                    holders["op"] = psum_op
                    pending = []
                    filler = pre_fill
                    filler += list(make_proj_units([4, 5], psum_op, "op", "op", "dve"))
                    filler += list(make_proj_units([6, 7], psum_op, "op", "op", "dve"))
                    for pi, (b, h, pr) in enumerate(passes):
                        if pi == 4:
                            # deferred projections must be fully emitted before
                            # any batch-1 read (emission order defines dataflow)
                            while filler:
                                filler.pop(0)()
                            filler = [
                                (lambda b0=0, j0=j: emit_outproj_tile(b0, j0))
                                for j in range(S // P)
                            ]
                        dsl = bass.ds(D * h, D)
                        av2 = psum_av.tile(
                            [D + 1, 2 * QCH], f32, tag="av", bufs=1, name="av"
                        )

                        def av_pair(t, pt):
                            lhs_v = Vtm[:, b * 16 + t, h * (D + 1) : (h + 1) * (D + 1)]
                            for qi in range(2):
                                nc.tensor.matmul(
                                    av2[:, bass.ts(qi, QCH)], lhs_v,
                                    pt[:, bass.ts(qi, QCH)],
                                    start=(t == 0), stop=(t == NKT - 1),
                                )

                        # software-pipelined one kt deep: scores(t) and exp(t)
                        # are issued before av(t-1), so the PE's av never waits
                        # on the exp it consumes
                        prev_pt = None
                        for t in range(NKT):
                            ksl = bass.ds(b * S + t * P, P)
                            lhs_k = KT[dsl, ksl]
                            sc2 = psum_sc.tile([P, 2 * QCH], f32, tag="sc", name="sc2")
                            for qi in range(2):
                                qc = 2 * pr + qi
                                qsl = bass.ds(b * S + qc * QCH, QCH)
                                nc.tensor.matmul(
                                    sc2[:, bass.ts(qi, QCH)], lhs_k, QT[dsl, qsl],
                                    start=True, stop=True,
                                )
                            pt = pt_pool.tile([P, 2 * QCH], f16, tag="pt", name="pt")
                            bt = b * 16 + t
                            nc.scalar.activation(
                                pt, sc2, Exp, bias=mask_sb[:, bt : bt + 1], scale=1.0
                            )
                            if prev_pt is not None:
                                av_pair(t - 1, prev_pt)
                            prev_pt = pt
                            # early-emit the previous pass's normalization
                            # (DVE/DMA only) so its reciprocal never gates PE
                            if t == 1 and pending:
                                emit_norm(*pending.pop(0))
                                if pi == 7:
                                    # batch-1 pair-0 columns are now normalized
                                    filler.extend(
                                        (lambda b1=1, j1=j: emit_outproj_tile(b1, j1))
                                        for j in range(S // (2 * P))
                                    )
                            # interleave independent PE work (deferred
                            # projections, ready out-proj tiles); cadence per
                            # pass keeps each pass near the exp() pace.
                            # pass 0 starts late (batch-1 x still landing);
                            # pass 7 keeps 4 tiles in reserve for the tail
                            # norm bubble
                            if filler:
                                if pi == 0:
                                    if t in (7, 10, 13, 15):
                                        filler.pop(0)()
                                elif pi < 4:
                                    if t in (3, 7, 11, 15):
                                        filler.pop(0)()
                                elif pi == 7:
                                    # keep ~7 pair-0 tiles in reserve: they
                                    # fill the PE during the tail norm's DRAM
                                    # round trip
                                    if t in (3, 9):
                                        filler.pop(0)()
                                elif t % 3 == 2:
                                    filler.pop(0)()
                        av_pair(NKT - 1, prev_pt)
                        # stage accumulators to SBUF, freeing the PSUM banks
                        av_sb = r_pool.tile(
                            [D + 1, 2 * QCH], f32, tag="avsb", bufs=3, name="avsb"
                        )
                        if pi == 7:
                            # last pass: denominator row staged separately on
                            # ACT (idle now) so the tail norm's DRAM bounce
                            # launches ~1.5us earlier
                            nc.scalar.copy(av_sb[D : D + 1, :], av2[D : D + 1, :])
                            nc.vector.tensor_copy(av_sb[0:D, :], av2[0:D, :])
                        else:
                            psum_to_sbuf(av_sb, av2)
                        pending.append((b, h, pr, av_sb))
                    # tail: drain remaining filler; interleave the last norm
                    # per-query-chunk with the out-proj tiles it unblocks
                    # tail: the last pass's denominator rows bounce straight
                    # from PSUM (av2 still live) so the broadcast RTT starts
                    # ~1.5us earlier; the reserved out-proj tiles fill the PE
                    # while the RTT is in flight
                    # tail order matters: both norm DMA bounces launch first,
                    # then the reserved out-proj tiles fill the PE (and the
                    # in-order DVE queue) while the DRAM round-trip is in
                    # flight, then the norms' DVE work and the final tiles
                    b_l, h_l, pr_l, av_sb_l = pending.pop(0)   # (1,1,1)
                    tail_mode[0] = True
                    sb0 = norm_dma(0, av_sb_l)
                    sb1 = norm_dma(1, av_sb_l)
                    while filler:
                        filler.pop(0)()
                    norm_dve(b_l, h_l, pr_l, 0, av_sb_l, sb0)
                    emit_outproj(1, 2 * S // (4 * P), 3 * S // (4 * P), spare_psum=True)
                    norm_dve(b_l, h_l, pr_l, 1, av_sb_l, sb1)
                    emit_outproj(1, 3 * S // (4 * P), S // P, spare_psum=True)

    nc.compile()
    return nc


def kernel(x, mask, Wq, bq, Wk, bk, Wv, bv, Wo, bo):
    global LAST_RESULTS, _PROGRAM
    from concourse.bass_utils import run_bass_kernel_spmd

    if _PROGRAM is None:
        _PROGRAM = _build_program()
    nc = _PROGRAM

    f16 = np.float16
    x = np.asarray(x, dtype=np.float32)
    mask = np.asarray(mask)
    f32c = lambda a: np.ascontiguousarray(np.asarray(a, dtype=np.float32))

    xT = np.ascontiguousarray(x.reshape(M, E).T.astype(f16))     # [E, M]
    maskf = np.where(mask, np.float32(NEG), np.float32(0.0)).astype(np.float32)
    maskT = np.ascontiguousarray(
        maskf.reshape(B, 16, P).transpose(2, 0, 1).reshape(P, B * 16)
    )
    scale = np.float32(1.0 / np.sqrt(D))

    in_maps = []
    for c in range(NCORES):
        csl = slice(P * c, P * (c + 1))
        wq_c = (np.asarray(Wq, dtype=np.float32)[:, csl] * scale).astype(f16)
        wk_c = np.asarray(Wk, dtype=np.float32)[:, csl].astype(f16)
        wv_c = np.asarray(Wv, dtype=np.float32)[:, csl].astype(f16)
        in_maps.append(
            {
                "xT": xT,
                "wq": np.ascontiguousarray(wq_c.reshape(KC, P, P).transpose(1, 0, 2)),
                "wk": np.ascontiguousarray(wk_c.reshape(KC, P, P).transpose(1, 0, 2)),
                "wv": np.ascontiguousarray(wv_c.reshape(KC, P, P).transpose(1, 0, 2)),
                "wo": np.ascontiguousarray(
                    np.asarray(Wo, dtype=np.float32)[csl, :].astype(f16)
                ),
                "bq": f32c(np.asarray(bq)[csl] * scale).reshape(P, 1),
                "bk": f32c(np.asarray(bk)[csl]).reshape(P, 1),
                "bv": f32c(np.asarray(bv)[csl]).reshape(P, 1),
                "maskT": maskT,
            }
        )

    trace = bool(os.environ.get("KERNEL_TRACE"))
    LAST_RESULTS = run_bass_kernel_spmd(
        nc, in_maps, list(range(NCORES)), trace=trace
    )

    acc = np.zeros((M, E), dtype=np.float64)
    for res in LAST_RESULTS.results:
        acc += res["out"].astype(np.float64)
    out = (acc + np.asarray(bo, dtype=np.float64)[None, :]).astype(np.float32)
    return out.reshape(B, S, E)
